# revision 14
# baseline (speedup 1.0000x reference)
"""Tensor-parallel GQA attention kernel for 8 Trainium2 NeuronCores.

Problem: x[2,2048,2048] -> Attention(16 q heads, 4 kv heads, rotary,
causal) -> out[2,2048,2048].

Sharding: core c handles batch b=c//4 and kv-group g=c%4 (4 q-heads +
1 kv-head). Each core computes its heads' attention output and a
partial O-projection [DIM, S] (output-dim major); the host sums the 4
partials per batch and transposes.

On-core dataflow (everything feature/dim-major so matmul contractions
land on the partition axis). All matmul operands are fp16 (fp32 PSUM
accumulation): fp16 gets fast-weight-load so LDWEIGHTS hides under the
previous matmul's stream, and runs 1 cycle/row at any free size.
  xT = transpose(x) via PE-transpose (fp16)
  QT/KT/VT = W.T @ xT
  RoPE applied per 512-chunk right after projection (overlaps PE work).
  Weight columns are pair-permuted on the host so partitions 0..63
  hold "real" dims, 64..127 "imag".
  scoresT[k,q] = KT_tile.T @ QT (pairs of k-tiles into one 2-bank
  PSUM tile) -> one exp per pair (ACT, ->fp16) -> mask (diag chunks)
  outT[dv,q] += V_tile.T @ attnT, sums[1,q] += ones.T @ attnT
  normalize via batched reciprocal_approx_fast + K=1 broadcast-matmul
  OT[o,q] += wo_tile.T @ outT
"""
import numpy as np

import concourse.bass as bass
import concourse.tile as tile
import concourse.mybir as mybir
from concourse import bacc
from concourse import bass_utils

F32 = mybir.dt.float32
F32R = mybir.dt.float32r
F16 = mybir.dt.float16

DIM = 2048
S = 2048
B = 2
HL = 4           # q heads per core
FT = DIM // 128  # feature tiles
TT = S // 128    # token tiles
CH = 4           # token chunks (512 tokens each) for projections
QC = 4           # q chunks (512) for attention
SCALE = 1.0 / np.sqrt(128.0)

_CACHE = {}


def _build():
    nc = bacc.Bacc("TRN2", target_bir_lowering=False, debug=False,
                   enable_asserts=True, num_devices=8)

    d_x = nc.dram_tensor("x_c", (S, DIM), F16, kind="ExternalInput").ap()
    d_wq = nc.dram_tensor("wq_c", (DIM, HL * 128), F16, kind="ExternalInput").ap()
    d_wk = nc.dram_tensor("wk_c", (DIM, 128), F16, kind="ExternalInput").ap()
    d_wv = nc.dram_tensor("wv_c", (DIM, 128), F16, kind="ExternalInput").ap()
    d_wo = nc.dram_tensor("wo_c", (HL * 128, DIM), F16, kind="ExternalInput").ap()
    d_cj = nc.dram_tensor("cjoin", (128, S), F16, kind="ExternalInput").ap()
    d_sj = nc.dram_tensor("sjoin", (128, S), F16, kind="ExternalInput").ap()
    d_mk = nc.dram_tensor("masks", (4, 128, 512), F16, kind="ExternalInput").ap()
    d_id = nc.dram_tensor("ident", (128, 128), F16, kind="ExternalInput").ap()
    d_ot = nc.dram_tensor("ot", (DIM, S), F32, kind="ExternalOutput").ap()

    Exp = mybir.ActivationFunctionType.Exp

    with tile.TileContext(nc) as tc:
        with tc.tile_pool(name="wts", bufs=1) as wp, \
             tc.tile_pool(name="acts", bufs=1) as ap:
            sb_id = wp.tile([128, 128], F16)
            nc.sync.dma_start(sb_id[:], d_id)
            sb_wq = wp.tile([128, FT, HL * 128], F16)
            nc.gpsimd.dma_start(sb_wq[:], d_wq.rearrange("(ft p) m -> p ft m", p=128))
            sb_wk = wp.tile([128, FT, 128], F16)
            nc.gpsimd.dma_start(sb_wk[:], d_wk.rearrange("(ft p) m -> p ft m", p=128))
            sb_wv = wp.tile([128, FT, 128], F16)
            nc.gpsimd.dma_start(sb_wv[:], d_wv.rearrange("(ft p) m -> p ft m", p=128))
            sb_cj = wp.tile([128, S], F16)
            nc.gpsimd.dma_start(sb_cj[:], d_cj)
            sb_sj = wp.tile([128, S], F16)
            nc.gpsimd.dma_start(sb_sj[:], d_sj)
            sb_mk = wp.tile([128, 4, 512], F16)
            nc.gpsimd.dma_start(sb_mk[:], d_mk.rearrange("m p n -> p m n"))
            sb_wo = wp.tile([128, HL, DIM], F16)
            nc.gpsimd.dma_start(sb_wo[:], d_wo.rearrange("(dv p) m -> p dv m", p=128))
            ones16 = wp.tile([128, 1], F16)
            nc.vector.memset(ones16[:], 1.0)
            ones32 = wp.tile([1, 128], F32)
            nc.vector.memset(ones32[:], 1.0)

            sb_QT = ap.tile([128, HL, S], F16)
            sb_KT = ap.tile([128, S], F16)
            sb_V = ap.tile([128, TT, 128], F16)
            sb_oT = ap.tile([128, HL, S], F16)

            # ---- Phase A: DMA-transpose x + Q/K/V projections + RoPE per chunk
            with tc.tile_pool(name="xT", bufs=3) as xT_p, \
                 tc.tile_pool(name="vt", bufs=2) as vt_p, \
                 tc.tile_pool(name="rope", bufs=2) as rp, \
                 tc.tile_pool(name="ps_tr", bufs=2, space="PSUM") as ps_tr, \
                 tc.tile_pool(name="ps_pj", bufs=3, space="PSUM") as ps_pj:

                def rope(T, c0):
                    # T: [128, 512] fp16 chunk at token offset c0
                    mc = rp.tile([128, 512], F16, tag="mc")
                    ms = rp.tile([128, 512], F16, tag="ms")
                    cjs = sb_cj[:, c0:c0 + 512]
                    sjs = sb_sj[:, c0:c0 + 512]
                    nc.gpsimd.tensor_mul(mc[:], T, cjs)
                    nc.vector.tensor_mul(ms[0:64, :], T[64:128, :], sjs[64:128, :])
                    nc.vector.tensor_mul(ms[64:128, :], T[0:64, :], sjs[0:64, :])
                    nc.vector.tensor_add(T, mc[:], ms[:])

                for ch in range(CH):
                    c0 = ch * 512
                    xt = xT_p.tile([128, FT, 512], F16)
                    for fi in range(FT):
                        nc.sync.dma_start(
                            xt[:, fi, :],
                            d_x[c0:c0 + 512, fi * 128:(fi + 1) * 128],
                            transpose=True)
                    for h in range(HL):
                        pq = ps_pj.tile([128, 512], F32, tag="pj")
                        for fi in range(FT):
                            nc.tensor.matmul(
                                pq[:], sb_wq[:, fi, h * 128:(h + 1) * 128],
                                xt[:, fi, :], start=(fi == 0), stop=(fi == FT - 1))
                        nc.vector.tensor_copy(sb_QT[:, h, c0:c0 + 512], pq[:])
                        rope(sb_QT[:, h, c0:c0 + 512], c0)
                    pk = ps_pj.tile([128, 512], F32, tag="pj")
                    for fi in range(FT):
                        nc.tensor.matmul(pk[:], sb_wk[:, fi, :], xt[:, fi, :],
                                         start=(fi == 0), stop=(fi == FT - 1))
                    nc.vector.tensor_copy(sb_KT[:, c0:c0 + 512], pk[:])
                    rope(sb_KT[:, c0:c0 + 512], c0)
                    pv = ps_pj.tile([128, 512], F32, tag="pj")
                    for fi in range(FT):
                        nc.tensor.matmul(pv[:], sb_wv[:, fi, :], xt[:, fi, :],
                                         start=(fi == 0), stop=(fi == FT - 1))
                    vt = vt_p.tile([128, 512], F16)
                    nc.vector.tensor_copy(vt[:], pv[:])
                    for tl in range(4):
                        ti = ch * 4 + tl
                        ptv = ps_tr.tile([128, 128], F16, tag="tr")
                        nc.tensor.transpose(
                            ptv[:], vt[:, tl * 128:(tl + 1) * 128], sb_id[:])
                        nc.vector.tensor_copy(sb_V[:, ti, :], ptv[:])

            # ---- Phase C: attention
            with tc.tile_pool(name="attn", bufs=6) as at_p, \
                 tc.tile_pool(name="bcst", bufs=2) as bc_p, \
                 tc.tile_pool(name="rcp", bufs=2) as rc_p, \
                 tc.tile_pool(name="ps_sc", bufs=2, space="PSUM") as ps_sc, \
                 tc.tile_pool(name="ps_o", bufs=2, space="PSUM") as ps_o, \
                 tc.tile_pool(name="ps_sum", bufs=1, space="PSUM") as ps_sum, \
                 tc.tile_pool(name="ps_bc", bufs=1, space="PSUM") as ps_bc:
                for qc in range(QC):
                    kmax = (qc + 1) * 4
                    q0 = qc * 512
                    for h in range(HL):
                        po = ps_o.tile([128, 512], F32, tag="po")
                        psum = ps_sum.tile([1, 512], F32, tag="ps")
                        for kp in range(kmax // 2):
                            psc = ps_sc.tile([128, 1024], F32, tag="sc")
                            at = at_p.tile([128, 1024], F16, tag="at")
                            for half in range(2):
                                ki = kp * 2 + half
                                nc.tensor.matmul(
                                    psc[:, half * 512:(half + 1) * 512],
                                    sb_KT[:, ki * 128:(ki + 1) * 128],
                                    sb_QT[:, h, q0:q0 + 512],
                                    start=True, stop=True)
                            nc.scalar.activation(at[:], psc[:], Exp, scale=SCALE)
                            for half in range(2):
                                ki = kp * 2 + half
                                if ki >= qc * 4:
                                    nc.vector.tensor_mul(
                                        at[:, half * 512:(half + 1) * 512],
                                        at[:, half * 512:(half + 1) * 512],
                                        sb_mk[:, ki - qc * 4, :])
                            for half in range(2):
                                ki = kp * 2 + half
                                nc.tensor.matmul(
                                    po[:], sb_V[:, ki, :],
                                    at[:, half * 512:(half + 1) * 512],
                                    start=(ki == 0), stop=(ki == kmax - 1))
                                nc.tensor.matmul(
                                    psum[:], ones16[:],
                                    at[:, half * 512:(half + 1) * 512],
                                    start=(ki == 0), stop=(ki == kmax - 1))
                        rc = rc_p.tile([1, 512], F32)
                        nc.vector.reciprocal_approx_fast(rc[:], psum[:])
                        pbc = ps_bc.tile([128, 512], F32, tag="bc")
                        nc.tensor.matmul(pbc[:], ones32[:], rc[:],
                                         start=True, stop=True)
                        bc = bc_p.tile([128, 512], F32)
                        nc.vector.tensor_copy(bc[:], pbc[:])
                        nc.vector.tensor_mul(
                            sb_oT[:, h, q0:q0 + 512], po[:], bc[:])

            # ---- Phase D: O projection
            with tc.tile_pool(name="otile", bufs=4) as ot_p, \
                 tc.tile_pool(name="ps_ot", bufs=4, space="PSUM") as ps_ot:
                for oi in range(FT):
                    for qc in range(QC):
                        pot = ps_ot.tile([128, 512], F32, tag="ot")
                        for dvi in range(HL):
                            nc.tensor.matmul(
                                pot[:], sb_wo[:, dvi, oi * 128:(oi + 1) * 128],
                                sb_oT[:, dvi, qc * 512:(qc + 1) * 512],
                                start=(dvi == 0), stop=(dvi == HL - 1))
                        otc = ot_p.tile([128, 512], F32)
                        if qc % 2 == 0:
                            nc.vector.tensor_copy(otc[:], pot[:])
                        else:
                            nc.scalar.copy(otc[:], pot[:])
                        nc.gpsimd.dma_start(
                            d_ot[oi * 128:(oi + 1) * 128,
                                 qc * 512:(qc + 1) * 512], otc[:])

    nc.compile()
    return nc


def _prep_shards(x, freqs_cos, freqs_sin, wq, wk, wv, wo):
    perm = np.empty(128, dtype=np.int64)
    perm[0:64] = 2 * np.arange(64)
    perm[64:128] = 2 * np.arange(64) + 1

    cosT = np.ascontiguousarray(freqs_cos.T).astype(np.float32)
    sinT = np.ascontiguousarray(freqs_sin.T).astype(np.float32)
    cjoin = np.concatenate([cosT, cosT], axis=0).astype(np.float16)
    sjoin = np.concatenate([sinT, -sinT], axis=0).astype(np.float16)

    masks = np.zeros((4, 128, 512), dtype=np.float16)
    q_idx = np.arange(512)[None, :]
    k_idx = np.arange(128)[:, None]
    for m in range(4):
        masks[m] = (q_idx >= m * 128 + k_idx).astype(np.float16)
    ident = np.eye(128, dtype=np.float16)

    in_maps = []
    for c in range(8):
        b, g = c // 4, c % 4
        wq_g = np.ascontiguousarray(
            wq[:, g * 512:(g + 1) * 512].reshape(DIM, 4, 128)[:, :, perm]
            .reshape(DIM, 512)).astype(np.float16)
        wk_g = np.ascontiguousarray(
            wk[:, g * 128:(g + 1) * 128][:, perm]).astype(np.float16)
        wv_g = np.ascontiguousarray(
            wv[:, g * 128:(g + 1) * 128]).astype(np.float16)
        wo_g = np.ascontiguousarray(
            wo[g * 512:(g + 1) * 512, :]).astype(np.float16)
        in_maps.append({
            "x_c": np.ascontiguousarray(x[b]).astype(np.float16),
            "wq_c": wq_g, "wk_c": wk_g, "wv_c": wv_g, "wo_c": wo_g,
            "cjoin": cjoin, "sjoin": sjoin, "masks": masks, "ident": ident,
        })
    return in_maps


def _assemble(results):
    out = np.zeros((B, S, DIM), dtype=np.float32)
    for c in range(8):
        out[c // 4] += results[c]["ot"].T
    return out


def kernel(x, freqs_cos, freqs_sin, wq, wk, wv, wo):
    x = np.asarray(x, dtype=np.float32)
    if "nc" not in _CACHE:
        _CACHE["nc"] = _build()
    nc = _CACHE["nc"]
    in_maps = _prep_shards(x, np.asarray(freqs_cos), np.asarray(freqs_sin),
                           np.asarray(wq), np.asarray(wk), np.asarray(wv),
                           np.asarray(wo))
    res = bass_utils.run_bass_kernel_spmd(nc, in_maps, core_ids=list(range(8)))
    return _assemble(res.results)


# revision 15
# speedup vs baseline: 1.1236x; 1.1236x over previous
"""Tensor-parallel GQA attention kernel for 8 Trainium2 NeuronCores.

Problem: x[2,2048,2048] -> Attention(16 q heads, 4 kv heads, rotary,
causal) -> out[2,2048,2048].

Sharding: core c handles batch b=c//4 and kv-group g=c%4 (4 q-heads +
1 kv-head). Each core computes its heads' attention output and a
partial O-projection [DIM, S] (output-dim major); the host sums the 4
partials per batch and transposes.

On-core dataflow (everything feature/dim-major so matmul contractions
land on the partition axis). All matmul operands are fp16 (fp32 PSUM
accumulation): fp16 gets fast-weight-load so LDWEIGHTS hides under the
previous matmul's stream, and runs 1 cycle/row at any free size.
  xT = transpose(x) via PE-transpose (fp16)
  QT/KT/VT = W.T @ xT
  RoPE applied per 512-chunk right after projection (overlaps PE work).
  Weight columns are pair-permuted on the host so partitions 0..63
  hold "real" dims, 64..127 "imag".
  scoresT[k,q] = KT_tile.T @ QT (pairs of k-tiles into one 2-bank
  PSUM tile) -> one exp per pair (ACT, ->fp16) -> mask (diag chunks)
  outT[dv,q] += V_tile.T @ attnT, sums[1,q] += ones.T @ attnT
  normalize via batched reciprocal_approx_fast + K=1 broadcast-matmul
  OT[o,q] += wo_tile.T @ outT
"""
import numpy as np

import concourse.bass as bass
import concourse.tile as tile
import concourse.mybir as mybir
from concourse import bacc
from concourse import bass_utils

F32 = mybir.dt.float32
F32R = mybir.dt.float32r
F16 = mybir.dt.float16

DIM = 2048
S = 2048
B = 2
HL = 4           # q heads per core
FT = DIM // 128  # feature tiles
TT = S // 128    # token tiles
CH = 4           # token chunks (512 tokens each) for projections
QC = 4           # q chunks (512) for attention
SCALE = 1.0 / np.sqrt(128.0)

_CACHE = {}


def _build():
    nc = bacc.Bacc("TRN2", target_bir_lowering=False, debug=False,
                   enable_asserts=True, num_devices=8)

    d_x = nc.dram_tensor("x_c", (S, DIM), F16, kind="ExternalInput").ap()
    d_wq = nc.dram_tensor("wq_c", (DIM, HL * 128), F16, kind="ExternalInput").ap()
    d_wk = nc.dram_tensor("wk_c", (DIM, 128), F16, kind="ExternalInput").ap()
    d_wv = nc.dram_tensor("wv_c", (DIM, 128), F16, kind="ExternalInput").ap()
    d_wo = nc.dram_tensor("wo_c", (HL * 128, DIM), F16, kind="ExternalInput").ap()
    d_cj = nc.dram_tensor("cjoin", (128, S), F16, kind="ExternalInput").ap()
    d_sj = nc.dram_tensor("sjoin", (128, S), F16, kind="ExternalInput").ap()
    d_mk = nc.dram_tensor("masks", (4, 128, 512), F16, kind="ExternalInput").ap()
    d_id = nc.dram_tensor("ident", (128, 128), F16, kind="ExternalInput").ap()
    d_ot = nc.dram_tensor("ot", (DIM, S), F32, kind="ExternalOutput").ap()

    Exp = mybir.ActivationFunctionType.Exp

    with tile.TileContext(nc) as tc:
        with tc.tile_pool(name="wts", bufs=1) as wp, \
             tc.tile_pool(name="acts", bufs=1) as ap:
            sb_id = wp.tile([128, 128], F16)
            nc.sync.dma_start(sb_id[:], d_id)
            sb_wq = wp.tile([128, FT, HL * 128], F16)
            nc.sync.dma_start(sb_wq[:], d_wq.rearrange("(ft p) m -> p ft m", p=128))
            sb_wk = wp.tile([128, FT, 128], F16)
            nc.sync.dma_start(sb_wk[:], d_wk.rearrange("(ft p) m -> p ft m", p=128))
            sb_wv = wp.tile([128, FT, 128], F16)
            nc.sync.dma_start(sb_wv[:], d_wv.rearrange("(ft p) m -> p ft m", p=128))
            sb_cj = wp.tile([128, S], F16)
            sb_sj = wp.tile([128, S], F16)
            sb_mk = wp.tile([128, 4, 512], F16)
            sb_wo = wp.tile([128, HL, DIM], F16)
            ones16 = wp.tile([128, 1], F16)
            nc.vector.memset(ones16[:], 1.0)
            ones32 = wp.tile([1, 128], F32)
            nc.vector.memset(ones32[:], 1.0)

            sb_QT = ap.tile([128, HL, S], F16)
            sb_KT = ap.tile([128, S], F16)
            sb_V = ap.tile([128, TT, 128], F16)
            sb_oT = ap.tile([128, HL, S], F16)

            # ---- Phase A: DMA-transpose x + Q/K/V projections + RoPE per chunk
            with tc.tile_pool(name="xT", bufs=2) as xT_p, \
                 tc.tile_pool(name="vt", bufs=2) as vt_p, \
                 tc.tile_pool(name="rope", bufs=2) as rp, \
                 tc.tile_pool(name="ps_tr", bufs=2, space="PSUM") as ps_tr, \
                 tc.tile_pool(name="ps_pj", bufs=3, space="PSUM") as ps_pj:

                def rope(T, c0):
                    # T: [128, 512] fp16 chunk at token offset c0
                    mc = rp.tile([128, 512], F16, tag="mc")
                    ms = rp.tile([128, 512], F16, tag="ms")
                    cjs = sb_cj[:, c0:c0 + 512]
                    sjs = sb_sj[:, c0:c0 + 512]
                    nc.gpsimd.tensor_mul(mc[:], T, cjs)
                    nc.vector.tensor_mul(ms[0:64, :], T[64:128, :], sjs[64:128, :])
                    nc.vector.tensor_mul(ms[64:128, :], T[0:64, :], sjs[0:64, :])
                    nc.vector.tensor_add(T, mc[:], ms[:])

                for ch in range(CH):
                    c0 = ch * 512
                    xt = xT_p.tile([128, FT, 512], F16)
                    for fi in range(FT):
                        nc.sync.dma_start(
                            xt[:, fi, :],
                            d_x[c0:c0 + 512, fi * 128:(fi + 1) * 128],
                            transpose=True)
                    if ch == 0:
                        nc.scalar.dma_start(sb_cj[:], d_cj)
                        nc.scalar.dma_start(sb_sj[:], d_sj)
                        nc.scalar.dma_start(sb_mk[:], d_mk.rearrange("m p n -> p m n"))
                        nc.scalar.dma_start(sb_wo[:], d_wo.rearrange("(dv p) m -> p dv m", p=128))
                    for h in range(HL):
                        pq = ps_pj.tile([128, 512], F32, tag="pj")
                        for fi in range(FT):
                            nc.tensor.matmul(
                                pq[:], sb_wq[:, fi, h * 128:(h + 1) * 128],
                                xt[:, fi, :], start=(fi == 0), stop=(fi == FT - 1))
                        nc.vector.tensor_copy(sb_QT[:, h, c0:c0 + 512], pq[:])
                        rope(sb_QT[:, h, c0:c0 + 512], c0)
                    pk = ps_pj.tile([128, 512], F32, tag="pj")
                    for fi in range(FT):
                        nc.tensor.matmul(pk[:], sb_wk[:, fi, :], xt[:, fi, :],
                                         start=(fi == 0), stop=(fi == FT - 1))
                    nc.vector.tensor_copy(sb_KT[:, c0:c0 + 512], pk[:])
                    rope(sb_KT[:, c0:c0 + 512], c0)
                    pv = ps_pj.tile([128, 512], F32, tag="pj")
                    for fi in range(FT):
                        nc.tensor.matmul(pv[:], sb_wv[:, fi, :], xt[:, fi, :],
                                         start=(fi == 0), stop=(fi == FT - 1))
                    vt = vt_p.tile([128, 512], F16)
                    nc.vector.tensor_copy(vt[:], pv[:])
                    for tl in range(4):
                        ti = ch * 4 + tl
                        ptv = ps_tr.tile([128, 128], F16, tag="tr")
                        nc.tensor.transpose(
                            ptv[:], vt[:, tl * 128:(tl + 1) * 128], sb_id[:])
                        nc.vector.tensor_copy(sb_V[:, ti, :], ptv[:])

            # ---- Phase C: attention
            with tc.tile_pool(name="attn", bufs=4) as at_p, \
                 tc.tile_pool(name="bcst", bufs=2) as bc_p, \
                 tc.tile_pool(name="rcp", bufs=2) as rc_p, \
                 tc.tile_pool(name="ps_sc", bufs=2, space="PSUM") as ps_sc, \
                 tc.tile_pool(name="ps_o", bufs=2, space="PSUM") as ps_o, \
                 tc.tile_pool(name="ps_sum", bufs=1, space="PSUM") as ps_sum, \
                 tc.tile_pool(name="ps_bc", bufs=1, space="PSUM") as ps_bc:
                for qc in range(QC):
                    kmax = (qc + 1) * 4
                    q0 = qc * 512
                    for h in range(HL):
                        po = ps_o.tile([128, 512], F32, tag="po")
                        psum = ps_sum.tile([1, 512], F32, tag="ps")
                        for kp in range(kmax // 2):
                            psc = ps_sc.tile([128, 1024], F32, tag="sc")
                            at = at_p.tile([128, 1024], F16, tag="at")
                            for half in range(2):
                                ki = kp * 2 + half
                                nc.tensor.matmul(
                                    psc[:, half * 512:(half + 1) * 512],
                                    sb_KT[:, ki * 128:(ki + 1) * 128],
                                    sb_QT[:, h, q0:q0 + 512],
                                    start=True, stop=True)
                            nc.scalar.activation(at[:], psc[:], Exp, scale=SCALE)
                            for half in range(2):
                                ki = kp * 2 + half
                                if ki >= qc * 4:
                                    nc.vector.tensor_mul(
                                        at[:, half * 512:(half + 1) * 512],
                                        at[:, half * 512:(half + 1) * 512],
                                        sb_mk[:, ki - qc * 4, :])
                            for half in range(2):
                                ki = kp * 2 + half
                                nc.tensor.matmul(
                                    po[:], sb_V[:, ki, :],
                                    at[:, half * 512:(half + 1) * 512],
                                    start=(ki == 0), stop=(ki == kmax - 1))
                                nc.tensor.matmul(
                                    psum[:], ones16[:],
                                    at[:, half * 512:(half + 1) * 512],
                                    start=(ki == 0), stop=(ki == kmax - 1))
                        rc = rc_p.tile([1, 512], F32)
                        nc.vector.reciprocal_approx_fast(rc[:], psum[:])
                        pbc = ps_bc.tile([128, 512], F32, tag="bc")
                        nc.tensor.matmul(pbc[:], ones32[:], rc[:],
                                         start=True, stop=True)
                        bc = bc_p.tile([128, 512], F32)
                        nc.vector.tensor_copy(bc[:], pbc[:])
                        nc.vector.tensor_mul(
                            sb_oT[:, h, q0:q0 + 512], po[:], bc[:])

            # ---- Phase D: O projection
            with tc.tile_pool(name="otile", bufs=4) as ot_p, \
                 tc.tile_pool(name="ps_ot", bufs=4, space="PSUM") as ps_ot:
                for oi in range(FT):
                    for qc in range(QC):
                        pot = ps_ot.tile([128, 512], F32, tag="ot")
                        for dvi in range(HL):
                            nc.tensor.matmul(
                                pot[:], sb_wo[:, dvi, oi * 128:(oi + 1) * 128],
                                sb_oT[:, dvi, qc * 512:(qc + 1) * 512],
                                start=(dvi == 0), stop=(dvi == HL - 1))
                        otc = ot_p.tile([128, 512], F32)
                        if qc % 2 == 0:
                            nc.vector.tensor_copy(otc[:], pot[:])
                        else:
                            nc.scalar.copy(otc[:], pot[:])
                        nc.sync.dma_start(
                            d_ot[oi * 128:(oi + 1) * 128,
                                 qc * 512:(qc + 1) * 512], otc[:])

    nc.compile()
    return nc


def _prep_shards(x, freqs_cos, freqs_sin, wq, wk, wv, wo):
    perm = np.empty(128, dtype=np.int64)
    perm[0:64] = 2 * np.arange(64)
    perm[64:128] = 2 * np.arange(64) + 1

    cosT = np.ascontiguousarray(freqs_cos.T).astype(np.float32)
    sinT = np.ascontiguousarray(freqs_sin.T).astype(np.float32)
    cjoin = np.concatenate([cosT, cosT], axis=0).astype(np.float16)
    sjoin = np.concatenate([sinT, -sinT], axis=0).astype(np.float16)

    masks = np.zeros((4, 128, 512), dtype=np.float16)
    q_idx = np.arange(512)[None, :]
    k_idx = np.arange(128)[:, None]
    for m in range(4):
        masks[m] = (q_idx >= m * 128 + k_idx).astype(np.float16)
    ident = np.eye(128, dtype=np.float16)

    in_maps = []
    for c in range(8):
        b, g = c // 4, c % 4
        wq_g = np.ascontiguousarray(
            wq[:, g * 512:(g + 1) * 512].reshape(DIM, 4, 128)[:, :, perm]
            .reshape(DIM, 512)).astype(np.float16)
        wk_g = np.ascontiguousarray(
            wk[:, g * 128:(g + 1) * 128][:, perm]).astype(np.float16)
        wv_g = np.ascontiguousarray(
            wv[:, g * 128:(g + 1) * 128]).astype(np.float16)
        wo_g = np.ascontiguousarray(
            wo[g * 512:(g + 1) * 512, :]).astype(np.float16)
        in_maps.append({
            "x_c": np.ascontiguousarray(x[b]).astype(np.float16),
            "wq_c": wq_g, "wk_c": wk_g, "wv_c": wv_g, "wo_c": wo_g,
            "cjoin": cjoin, "sjoin": sjoin, "masks": masks, "ident": ident,
        })
    return in_maps


def _assemble(results):
    out = np.zeros((B, S, DIM), dtype=np.float32)
    for c in range(8):
        out[c // 4] += results[c]["ot"].T
    return out


def kernel(x, freqs_cos, freqs_sin, wq, wk, wv, wo):
    x = np.asarray(x, dtype=np.float32)
    if "nc" not in _CACHE:
        _CACHE["nc"] = _build()
    nc = _CACHE["nc"]
    in_maps = _prep_shards(x, np.asarray(freqs_cos), np.asarray(freqs_sin),
                           np.asarray(wq), np.asarray(wk), np.asarray(wv),
                           np.asarray(wo))
    res = bass_utils.run_bass_kernel_spmd(nc, in_maps, core_ids=list(range(8)))
    return _assemble(res.results)
